# revision 3
# baseline (speedup 1.0000x reference)
"""Trainium2 Bass kernel for the autoregressive LSTM decoder problem.

Full-input contract: kernel(**inputs) takes the unsharded numpy inputs
(B=8192, D=512, K=24) and returns (out1, out2), each [B, K] float32.

Strategy (data-parallel over 8 NeuronCores, B/8 = 1024 batch per core):
  * All state is kept transposed on-chip: h,c as [D, B_shard] so the
    per-step gate matmul z^T = Wu^T @ h^T + Wx^T x^T lands in PSUM already
    gate-major; no transposes anywhere in the decode loop.
  * Matmul operands are bf16 (fp32 PSUM accumulation); all elementwise
    math is fp32. The LSTM's saturating gates keep the bf16 matmul noise
    ~2e-4 (y1) / 3e-3 (y2) scale-relative at the outputs.
  * The rank-1 x@Wx term closes each 5-matmul PSUM accumulation group as a
    K=1 matmul; the four closers of a k-group go to four distinct 32-row PE
    tile_positions back-to-back so they run concurrently on the PE array.
  * ScalarE applies sigmoid/tanh directly PSUM->SBUF (fused evacuation);
    VectorE does the c/h updates; y2's elu is deferred and applied once,
    batched [K, B_shard], after the step loop.
"""

import sys

import numpy as np

for _p in ("/opt/trn_rl_repo", "/root/.axon_site/_ro/trn_rl_repo"):
    if _p not in sys.path:
        sys.path.append(_p)

import concourse.bass as bass
import concourse.mybir as mybir
from concourse.tile import TileContext
from concourse.vector_clock import ScopedClock

F32 = mybir.dt.float32
BF16 = mybir.dt.bfloat16
AF = mybir.ActivationFunctionType

D = 512
B = 1024          # batch per core
NCORES = 8
K = 24
G = 4 * D         # 2048 gate rows
KT = D // 128     # 4 k tiles
MT = G // 128     # 16 gate row tiles
NT = B // 512     # 2 batch chunks
N = 512

_MAX_WAITS_PER_DRAIN = 1


def _split_waits(nc):
    """The walrus build in this container accepts at most one semaphore wait
    per instruction. Rebuild every basic block, hoisting all-but-one wait of
    any overloaded instruction onto same-engine InstEventSemaphore
    instructions inserted immediately before it — the engine blocks at the
    same program point for the same conditions, so this is
    semantics-preserving."""
    n_new = 0
    for f in nc.m.functions:
        for blk in f.blocks:
            insts = list(blk.instructions)
            out = []
            changed = False
            for inst in insts:
                si = inst.sync_info
                waits = list(si.on_wait) if si is not None else []
                if len(waits) > 1:
                    changed = True
                    excess, keep = waits[:-1], waits[-1:]
                    for w in excess:
                        ev = mybir.InstEventSemaphore(
                            name=f"splitw-{n_new}", ins=[], outs=[],
                            engine=inst.engine,
                        )
                        ev.sync_info = mybir.SyncInfo(on_wait=[w], on_update=[])
                        nc.register_instruction(ev, overwrite=True)
                        out.append(ev)
                        n_new += 1
                    inst.sync_info = mybir.SyncInfo(
                        on_wait=keep, on_update=list(si.on_update)
                    )
                out.append(inst)
            if changed:
                blk.instructions = out
    return n_new


class SplitDrainTileContext(TileContext):
    """The walrus build in this container rejects Drain (CTRL_NO)
    instructions carrying more than ~2 sync waits; split the tail drain's
    waits across a chain of Drain instructions, one wait each."""

    def _drain_and_barrier(self, tick_clock, wait_clock):
        nc = self.nc
        drain_inst = nc.sync.drain()
        wait_clock.add_sem_waits(
            drain_inst.ins, ScopedClock({None: tick_clock.global_clock})
        )
        si = drain_inst.ins.sync_info
        waits = list(si.on_wait) if si is not None else []
        if len(waits) > _MAX_WAITS_PER_DRAIN:
            drain_inst.ins.sync_info = mybir.SyncInfo(
                on_wait=waits[:_MAX_WAITS_PER_DRAIN], on_update=[]
            )
            for i in range(_MAX_WAITS_PER_DRAIN, len(waits), _MAX_WAITS_PER_DRAIN):
                extra = nc.sync.drain()
                extra.ins.sync_info = mybir.SyncInfo(
                    on_wait=waits[i : i + _MAX_WAITS_PER_DRAIN], on_update=[]
                )

        nc.all_engine_barrier()
        assert self.sems is not None
        popped = nc._tile_sem_poison_stack.pop()
        assert popped is self._sem_poison
        nc.clear_and_free_semaphores(list(self.sems.allocated().values()))
        nc.all_engine_barrier()


def build_nc(repeat: int = 0, rowtile_wx: bool = True):
    """repeat=0: straight-line kernel. repeat>=1: whole body wrapped in a
    For_i loop run `repeat` times (only used for timing measurements)."""
    import contextlib

    nc = bass.Bass()

    hT0 = nc.dram_tensor("hT0", [D, B], BF16, kind="ExternalInput")
    cT0 = nc.dram_tensor("cT0", [D, B], F32, kind="ExternalInput")
    wu = nc.dram_tensor("wu", [D, G], BF16, kind="ExternalInput")
    wx = nc.dram_tensor("wx", [4, G], BF16, kind="ExternalInput")
    w12 = nc.dram_tensor("w12", [D, 2], BF16, kind="ExternalInput")
    bvec = nc.dram_tensor("bvec", [G], F32, kind="ExternalInput")
    b12 = nc.dram_tensor("b12", [2, 1], F32, kind="ExternalInput")
    b2col = nc.dram_tensor("b2col", [K, 1], F32, kind="ExternalInput")
    x0 = nc.dram_tensor("x0", [1, B], BF16, kind="ExternalInput")
    ys1 = nc.dram_tensor("ys1", [K, B], F32, kind="ExternalOutput")
    ys2 = nc.dram_tensor("ys2", [K, B], F32, kind="ExternalOutput")

    with SplitDrainTileContext(nc) as tc:
        with contextlib.ExitStack() as ctx:
            wpool = ctx.enter_context(tc.tile_pool(name="w", bufs=1))
            hpool = ctx.enter_context(tc.tile_pool(name="h", bufs=16))
            cpool = ctx.enter_context(tc.tile_pool(name="c", bufs=16))
            gpool = ctx.enter_context(tc.tile_pool(name="g", bufs=16))
            tpool = ctx.enter_context(tc.tile_pool(name="t", bufs=4))
            xpool = ctx.enter_context(tc.tile_pool(name="x", bufs=3))
            ypool = ctx.enter_context(tc.tile_pool(name="y", bufs=4))
            opool = ctx.enter_context(tc.tile_pool(name="o", bufs=1))
            zps = ctx.enter_context(tc.tile_pool(name="zp", bufs=6, space="PSUM"))
            yps = ctx.enter_context(tc.tile_pool(name="yp", bufs=2, space="PSUM"))

            loop_cm = tc.For_i(0, repeat) if repeat else contextlib.nullcontext()
            with loop_cm:
                # --- weights + state init -------------------------------
                wu_sb = wpool.tile([128, KT * G], BF16, tag="wu")
                for k in range(KT):
                    nc.sync.dma_start(
                        wu_sb[:, k * G:(k + 1) * G], wu[k * 128:(k + 1) * 128, :]
                    )
                wx_sb = wpool.tile([128, G], BF16, tag="wx")
                w12_sb = wpool.tile([128, KT * 2], BF16, tag="w12")
                for k in range(KT):
                    nc.sync.dma_start(
                        w12_sb[:, 2 * k:2 * k + 2], w12[k * 128:(k + 1) * 128, :]
                    )
                b_sb = wpool.tile([128, MT], F32, tag="b")
                nc.sync.dma_start(b_sb[:, :], bvec[:].rearrange("(m p) -> p m", p=128))
                b12_sb = wpool.tile([2, 1], F32, tag="b12")
                nc.sync.dma_start(b12_sb[:, :], b12[:, :])
                b2c_sb = wpool.tile([K, 1], F32, tag="b2col")
                nc.sync.dma_start(b2c_sb[:, :], b2col[:, :])

                h_prev, c_prev = {}, {}
                for k in range(KT):
                    for n in range(NT):
                        ht = hpool.tile([128, N], BF16, tag="h")
                        nc.sync.dma_start(
                            ht[:, :], hT0[k * 128:(k + 1) * 128, n * N:(n + 1) * N]
                        )
                        h_prev[(k, n)] = ht
                        ct = cpool.tile([128, N], F32, tag="c")
                        nc.sync.dma_start(
                            ct[:, :], cT0[k * 128:(k + 1) * 128, n * N:(n + 1) * N]
                        )
                        c_prev[(k, n)] = ct
                x_prev = xpool.tile([128, B], BF16, tag="x")
                nrows = 4 if rowtile_wx else 1
                for j in range(nrows):
                    nc.sync.dma_start(wx_sb[32 * j:32 * j + 1, :], wx[j:j + 1, :])
                    nc.sync.dma_start(x_prev[32 * j:32 * j + 1, :], x0[0:1, :])

                ys2pre = opool.tile([K, B], F32, tag="ys2pre")

                # --- decode steps ---------------------------------------
                for t in range(K):
                    x_next = xpool.tile([128, B], BF16, tag="x")
                    h_new, c_new = {}, {}
                    for n in range(NT):
                        ns = slice(n * N, (n + 1) * N)
                        for k in range(KT):
                            gates = (k, 4 + k, 8 + k, 12 + k)
                            zp_m = {}
                            for m in gates:
                                zp = zps.tile([128, N], F32, tag="z")
                                for kk in range(KT):
                                    nc.tensor.matmul(
                                        zp[:, :],
                                        wu_sb[:, kk * G + m * 128:kk * G + (m + 1) * 128],
                                        h_prev[(kk, n)][:, :],
                                        start=(kk == 0),
                                        stop=False,
                                    )
                                zp_m[m] = zp
                            # rank-1 x closers, back-to-back on 4 row groups
                            for m in gates:
                                j = (m // 4) if rowtile_wx else 0
                                nc.tensor.matmul(
                                    zp_m[m][:, :],
                                    wx_sb[32 * j:32 * j + 1, m * 128:(m + 1) * 128],
                                    x_prev[32 * j:32 * j + 1, ns],
                                    start=False,
                                    stop=True,
                                    tile_position=(32 * j, 0) if rowtile_wx else None,
                                )
                            gt = {}
                            for m in gates:
                                g = gpool.tile([128, N], F32, tag="g")
                                func = AF.Tanh if m // 4 == 2 else AF.Sigmoid
                                nc.scalar.activation(
                                    g[:, :], zp_m[m][:, :], func, bias=b_sb[:, m:m + 1]
                                )
                                gt[m] = g
                            t1 = tpool.tile([128, N], F32, tag="t1")
                            nc.vector.tensor_mul(t1[:, :], gt[4 + k][:, :], c_prev[(k, n)][:, :])
                            t2 = tpool.tile([128, N], F32, tag="t2")
                            nc.vector.tensor_mul(t2[:, :], gt[k][:, :], gt[8 + k][:, :])
                            cn = cpool.tile([128, N], F32, tag="c")
                            nc.vector.tensor_add(cn[:, :], t1[:, :], t2[:, :])
                            tch = tpool.tile([128, N], F32, tag="tch")
                            nc.scalar.activation(tch[:, :], cn[:, :], AF.Tanh)
                            hn = hpool.tile([128, N], BF16, tag="h")
                            nc.vector.tensor_mul(hn[:, :], gt[12 + k][:, :], tch[:, :])
                            h_new[(k, n)] = hn
                            c_new[(k, n)] = cn
                        # y head for chunk n
                        yp = yps.tile([2, N], F32, tag="y")
                        for k in range(KT):
                            nc.tensor.matmul(
                                yp[:, :],
                                w12_sb[:, 2 * k:2 * k + 2],
                                h_new[(k, n)][:, :],
                                start=(k == 0),
                                stop=(k == KT - 1),
                            )
                        yr1 = ypool.tile([2, N], F32, tag="yr1")
                        nc.scalar.activation(
                            yr1[0:1, :], yp[0:1, :], AF.Sigmoid, bias=b12_sb[0:1, 0:1]
                        )
                        nc.sync.dma_start(ys1[t:t + 1, ns], yr1[0:1, :])
                        nc.scalar.activation(
                            x_next[0:1, ns], yp[0:1, :], AF.Sigmoid,
                            bias=b12_sb[0:1, 0:1],
                        )
                        yr2 = ypool.tile([2, N], F32, tag="yr2")
                        nc.scalar.copy(yr2[0:2, :], yp[0:2, :])
                        nc.sync.dma_start(ys2pre[t:t + 1, ns], yr2[1:2, :])
                    if rowtile_wx:
                        for j in range(1, 4):
                            nc.sync.dma_start(
                                x_next[32 * j:32 * j + 1, :], x_next[0:1, :]
                            )
                    h_prev, c_prev, x_prev = h_new, c_new, x_next

                # --- batched elu tail: y2 = relu(p) + exp(min(p,0)) - 1 --
                pb = opool.tile([K, B], F32, tag="elu_p")
                nc.scalar.activation(
                    pb[:, :], ys2pre[:, :], AF.Identity, bias=b2c_sb[:, 0:1]
                )
                r = opool.tile([K, B], F32, tag="elu_r")
                nc.scalar.activation(r[:, :], pb[:, :], AF.Relu)
                neg = opool.tile([K, B], F32, tag="elu_n")
                nc.vector.tensor_sub(neg[:, :], pb[:, :], r[:, :])
                e = opool.tile([K, B], F32, tag="elu_e")
                nc.scalar.activation(e[:, :], neg[:, :], AF.Exp)
                s = opool.tile([K, B], F32, tag="elu_s")
                nc.vector.tensor_add(s[:, :], r[:, :], e[:, :])
                y2f = opool.tile([K, B], F32, tag="elu_y")
                nc.vector.tensor_scalar_add(y2f[:, :], s[:, :], -1.0)
                nc.sync.dma_start(ys2[:, :], y2f[:, :])

    _split_waits(nc)
    return nc


def make_in_map(initial, encoder_hidden, encoder_cell, Wx, Wu, b, w1, b1, w2, b2):
    """Per-core input dict from this core's batch shard (numpy fp32 arrays)."""
    import ml_dtypes
    bf = lambda a: np.ascontiguousarray(a).astype(ml_dtypes.bfloat16)
    f32 = lambda a: np.ascontiguousarray(a, dtype=np.float32)
    return {
        "hT0": bf(encoder_hidden.T),
        "cT0": f32(encoder_cell.T),
        "wu": bf(Wu),
        "wx": bf(np.broadcast_to(Wx, (4, G))),
        "w12": bf(np.concatenate([w1, w2], axis=1)),
        "bvec": f32(b),
        "b12": np.array([[np.float32(b1[0])], [np.float32(b2[0])]], dtype=np.float32),
        "b2col": np.full((K, 1), np.float32(b2[0]), dtype=np.float32),
        "x0": bf(initial[:, 0, :].T),
    }


_CACHE = {}


def _get_nc():
    if "nc" not in _CACHE:
        _CACHE["nc"] = build_nc(repeat=0, rowtile_wx=True)
    return _CACHE["nc"]


def kernel(initial, encoder_hidden, encoder_cell, Wx, Wu, b, w1, b1, w2, b2):
    from concourse import bass_utils

    initial = np.asarray(initial, dtype=np.float32)
    encoder_hidden = np.asarray(encoder_hidden, dtype=np.float32)
    encoder_cell = np.asarray(encoder_cell, dtype=np.float32)
    Wx = np.asarray(Wx, dtype=np.float32)
    Wu = np.asarray(Wu, dtype=np.float32)
    b = np.asarray(b, dtype=np.float32)
    w1 = np.asarray(w1, dtype=np.float32)
    b1 = np.asarray(b1, dtype=np.float32)
    w2 = np.asarray(w2, dtype=np.float32)
    b2 = np.asarray(b2, dtype=np.float32)

    nc = _get_nc()
    in_maps = []
    for c in range(NCORES):
        sl = slice(c * B, (c + 1) * B)
        in_maps.append(
            make_in_map(initial[sl], encoder_hidden[sl], encoder_cell[sl],
                        Wx, Wu, b, w1, b1, w2, b2)
        )
    res = bass_utils.run_bass_kernel_spmd(nc, in_maps, core_ids=list(range(NCORES)))
    out1 = np.concatenate([res.results[c]["ys1"].T for c in range(NCORES)], axis=0)
    out2 = np.concatenate([res.results[c]["ys2"].T for c in range(NCORES)], axis=0)
    return (np.ascontiguousarray(out1, dtype=np.float32),
            np.ascontiguousarray(out2, dtype=np.float32))


# revision 4
# speedup vs baseline: 2.4596x; 2.4596x over previous
"""Trainium2 Bass kernel for the autoregressive LSTM decoder problem.

Full-input contract: kernel(**inputs) takes the unsharded numpy inputs
(B=8192, D=512, K=24) and returns (out1, out2), each [B, K] float32.

Strategy (data-parallel over 8 NeuronCores, B/8 = 1024 batch per core):
  * All state is kept transposed on-chip: h,c as [D, B_shard] so the
    per-step gate matmul z^T = Wu^T @ h^T + Wx^T x^T lands in PSUM already
    gate-major; no transposes anywhere in the decode loop.
  * Matmul operands are bf16 (fp32 PSUM accumulation); all elementwise
    math is fp32. The LSTM's saturating gates keep the bf16 matmul noise
    ~2e-4 (y1) / 3e-3 (y2) scale-relative at the outputs.
  * The rank-1 x@Wx term closes each 5-matmul PSUM accumulation group as a
    K=1 matmul; the four closers of a k-group go to four distinct 32-row PE
    tile_positions back-to-back so they run concurrently on the PE array.
  * ScalarE applies sigmoid/tanh directly PSUM->SBUF (fused evacuation);
    VectorE does the c/h updates; y2's elu is deferred and applied once,
    batched [K, B_shard], after the step loop.
"""

import sys

import numpy as np

for _p in ("/opt/trn_rl_repo", "/root/.axon_site/_ro/trn_rl_repo"):
    if _p not in sys.path:
        sys.path.append(_p)

import concourse.bass as bass
import concourse.mybir as mybir
from concourse.tile import TileContext
from concourse.vector_clock import ScopedClock

F32 = mybir.dt.float32
BF16 = mybir.dt.bfloat16
AF = mybir.ActivationFunctionType

D = 512
B = 1024          # batch per core
NCORES = 8
K = 24
G = 4 * D         # 2048 gate rows
KT = D // 128     # 4 k tiles
MT = G // 128     # 16 gate row tiles
NT = B // 512     # 2 batch chunks
N = 512

_MAX_WAITS_PER_DRAIN = 1


def _split_waits(nc):
    """The walrus build in this container accepts at most one semaphore wait
    per instruction. Rebuild every basic block, hoisting all-but-one wait of
    any overloaded instruction onto same-engine InstEventSemaphore
    instructions inserted immediately before it — the engine blocks at the
    same program point for the same conditions, so this is
    semantics-preserving."""
    n_new = 0
    for f in nc.m.functions:
        for blk in f.blocks:
            insts = list(blk.instructions)
            out = []
            changed = False
            for inst in insts:
                si = inst.sync_info
                waits = list(si.on_wait) if si is not None else []
                if len(waits) > 1:
                    changed = True
                    excess, keep = waits[:-1], waits[-1:]
                    for w in excess:
                        ev = mybir.InstEventSemaphore(
                            name=f"splitw-{n_new}", ins=[], outs=[],
                            engine=inst.engine,
                        )
                        ev.sync_info = mybir.SyncInfo(on_wait=[w], on_update=[])
                        nc.register_instruction(ev, overwrite=True)
                        out.append(ev)
                        n_new += 1
                    inst.sync_info = mybir.SyncInfo(
                        on_wait=keep, on_update=list(si.on_update)
                    )
                out.append(inst)
            if changed:
                blk.instructions = out
    return n_new


class SplitDrainTileContext(TileContext):
    """The walrus build in this container rejects Drain (CTRL_NO)
    instructions carrying more than ~2 sync waits; split the tail drain's
    waits across a chain of Drain instructions, one wait each."""

    def _drain_and_barrier(self, tick_clock, wait_clock):
        nc = self.nc
        drain_inst = nc.sync.drain()
        wait_clock.add_sem_waits(
            drain_inst.ins, ScopedClock({None: tick_clock.global_clock})
        )
        si = drain_inst.ins.sync_info
        waits = list(si.on_wait) if si is not None else []
        if len(waits) > _MAX_WAITS_PER_DRAIN:
            drain_inst.ins.sync_info = mybir.SyncInfo(
                on_wait=waits[:_MAX_WAITS_PER_DRAIN], on_update=[]
            )
            for i in range(_MAX_WAITS_PER_DRAIN, len(waits), _MAX_WAITS_PER_DRAIN):
                extra = nc.sync.drain()
                extra.ins.sync_info = mybir.SyncInfo(
                    on_wait=waits[i : i + _MAX_WAITS_PER_DRAIN], on_update=[]
                )

        nc.all_engine_barrier()
        assert self.sems is not None
        popped = nc._tile_sem_poison_stack.pop()
        assert popped is self._sem_poison
        nc.clear_and_free_semaphores(list(self.sems.allocated().values()))
        nc.all_engine_barrier()


def build_nc(repeat: int = 0, rowtile_wx: bool = True):
    """repeat=0: straight-line kernel. repeat>=1: whole body wrapped in a
    For_i loop run `repeat` times (only used for timing measurements)."""
    import contextlib

    nc = bass.Bass()

    hT0 = nc.dram_tensor("hT0", [D, B], BF16, kind="ExternalInput")
    cT0 = nc.dram_tensor("cT0", [D, B], F32, kind="ExternalInput")
    wu = nc.dram_tensor("wu", [D, G], BF16, kind="ExternalInput")
    wx = nc.dram_tensor("wx", [4, G], BF16, kind="ExternalInput")
    w12 = nc.dram_tensor("w12", [D, 2], BF16, kind="ExternalInput")
    bvec = nc.dram_tensor("bvec", [G], F32, kind="ExternalInput")
    b12 = nc.dram_tensor("b12", [2, 1], F32, kind="ExternalInput")
    b2col = nc.dram_tensor("b2col", [K, 1], F32, kind="ExternalInput")
    x0 = nc.dram_tensor("x0", [1, B], BF16, kind="ExternalInput")
    ys1 = nc.dram_tensor("ys1", [K, B], F32, kind="ExternalOutput")
    ys2 = nc.dram_tensor("ys2", [K, B], F32, kind="ExternalOutput")

    with SplitDrainTileContext(nc) as tc:
        with contextlib.ExitStack() as ctx:
            wpool = ctx.enter_context(tc.tile_pool(name="w", bufs=1))
            hpool = ctx.enter_context(tc.tile_pool(name="h", bufs=16))
            cpool = ctx.enter_context(tc.tile_pool(name="c", bufs=16))
            gpool = ctx.enter_context(tc.tile_pool(name="g", bufs=16))
            tpool = ctx.enter_context(tc.tile_pool(name="t", bufs=4))
            xpool = ctx.enter_context(tc.tile_pool(name="x", bufs=3))
            ypool = ctx.enter_context(tc.tile_pool(name="y", bufs=4))
            opool = ctx.enter_context(tc.tile_pool(name="o", bufs=1))
            zps = ctx.enter_context(tc.tile_pool(name="zp", bufs=6, space="PSUM"))
            yps = ctx.enter_context(tc.tile_pool(name="yp", bufs=2, space="PSUM"))

            loop_cm = tc.For_i(0, repeat) if repeat else contextlib.nullcontext()
            with loop_cm:
                # --- weights + state init -------------------------------
                wu_sb = wpool.tile([128, KT * G], BF16, tag="wu")
                for k in range(KT):
                    nc.sync.dma_start(
                        wu_sb[:, k * G:(k + 1) * G], wu[k * 128:(k + 1) * 128, :]
                    )
                wx_sb = wpool.tile([128, G], BF16, tag="wx")
                w12_sb = wpool.tile([128, KT * 2], BF16, tag="w12")
                for k in range(KT):
                    nc.sync.dma_start(
                        w12_sb[:, 2 * k:2 * k + 2], w12[k * 128:(k + 1) * 128, :]
                    )
                b_sb = wpool.tile([128, MT], F32, tag="b")
                nc.sync.dma_start(b_sb[:, :], bvec[:].rearrange("(m p) -> p m", p=128))
                b12_sb = wpool.tile([2, 1], F32, tag="b12")
                nc.sync.dma_start(b12_sb[:, :], b12[:, :])
                b2c_sb = wpool.tile([K, 1], F32, tag="b2col")
                nc.sync.dma_start(b2c_sb[:, :], b2col[:, :])

                h_prev, c_prev = {}, {}
                for k in range(KT):
                    for n in range(NT):
                        ht = hpool.tile([128, N], BF16, tag="h")
                        nc.sync.dma_start(
                            ht[:, :], hT0[k * 128:(k + 1) * 128, n * N:(n + 1) * N]
                        )
                        h_prev[(k, n)] = ht
                        ct = cpool.tile([128, N], F32, tag="c")
                        nc.sync.dma_start(
                            ct[:, :], cT0[k * 128:(k + 1) * 128, n * N:(n + 1) * N]
                        )
                        c_prev[(k, n)] = ct
                x_prev = xpool.tile([128, B], BF16, tag="x")
                nrows = 4 if rowtile_wx else 1
                for j in range(nrows):
                    nc.sync.dma_start(wx_sb[32 * j:32 * j + 1, :], wx[j:j + 1, :])
                    nc.sync.dma_start(x_prev[32 * j:32 * j + 1, :], x0[0:1, :])

                ys2pre = opool.tile([K, B], F32, tag="ys2pre")

                # --- decode steps ---------------------------------------
                for t in range(K):
                    x_next = xpool.tile([128, B], BF16, tag="x")
                    h_new, c_new = {}, {}
                    for n in range(NT):
                        ns = slice(n * N, (n + 1) * N)
                        for k in range(KT):
                            gates = (k, 4 + k, 8 + k, 12 + k)
                            zp_m = {}
                            for m in gates:
                                zp = zps.tile([128, N], F32, tag="z")
                                for kk in range(KT):
                                    nc.tensor.matmul(
                                        zp[:, :],
                                        wu_sb[:, kk * G + m * 128:kk * G + (m + 1) * 128],
                                        h_prev[(kk, n)][:, :],
                                        start=(kk == 0),
                                        stop=False,
                                    )
                                zp_m[m] = zp
                            # rank-1 x closers, back-to-back on 4 row groups
                            for m in gates:
                                j = (m // 4) if rowtile_wx else 0
                                nc.tensor.matmul(
                                    zp_m[m][:, :],
                                    wx_sb[32 * j:32 * j + 1, m * 128:(m + 1) * 128],
                                    x_prev[32 * j:32 * j + 1, ns],
                                    start=False,
                                    stop=True,
                                    tile_position=(32 * j, 0) if rowtile_wx else None,
                                )
                            gt = {}
                            for m in gates:
                                g = gpool.tile([128, N], F32, tag="g")
                                func = AF.Tanh if m // 4 == 2 else AF.Sigmoid
                                nc.scalar.activation(
                                    g[:, :], zp_m[m][:, :], func, bias=b_sb[:, m:m + 1]
                                )
                                gt[m] = g
                            t1 = tpool.tile([128, N], F32, tag="t1")
                            nc.vector.tensor_mul(t1[:, :], gt[4 + k][:, :], c_prev[(k, n)][:, :])
                            t2 = tpool.tile([128, N], F32, tag="t2")
                            nc.vector.tensor_mul(t2[:, :], gt[k][:, :], gt[8 + k][:, :])
                            cn = cpool.tile([128, N], F32, tag="c")
                            nc.vector.tensor_add(cn[:, :], t1[:, :], t2[:, :])
                            tch = tpool.tile([128, N], F32, tag="tch")
                            nc.scalar.activation(tch[:, :], cn[:, :], AF.Tanh)
                            hn = hpool.tile([128, N], BF16, tag="h")
                            nc.vector.tensor_mul(hn[:, :], gt[12 + k][:, :], tch[:, :])
                            h_new[(k, n)] = hn
                            c_new[(k, n)] = cn
                        # y head for chunk n
                        yp = yps.tile([2, N], F32, tag="y")
                        for k in range(KT):
                            nc.tensor.matmul(
                                yp[:, :],
                                w12_sb[:, 2 * k:2 * k + 2],
                                h_new[(k, n)][:, :],
                                start=(k == 0),
                                stop=(k == KT - 1),
                            )
                        yr1 = ypool.tile([2, N], F32, tag="yr1")
                        nc.scalar.activation(
                            yr1[0:1, :], yp[0:1, :], AF.Sigmoid, bias=b12_sb[0:1, 0:1]
                        )
                        nc.sync.dma_start(ys1[t:t + 1, ns], yr1[0:1, :])
                        yr2 = ypool.tile([2, N], F32, tag="yr2")
                        # x feedback and the y2 row copy ride VectorE to keep
                        # ScalarE (the second-busiest engine) lean
                        nc.vector.tensor_copy(x_next[0:1, ns], yr1[0:1, :])
                        nc.vector.tensor_copy(yr2[0:2, :], yp[0:2, :])
                        nc.sync.dma_start(ys2pre[t:t + 1, ns], yr2[1:2, :])
                    if rowtile_wx:
                        for j in range(1, 4):
                            nc.sync.dma_start(
                                x_next[32 * j:32 * j + 1, :], x_next[0:1, :]
                            )
                    h_prev, c_prev, x_prev = h_new, c_new, x_next

                # --- batched elu tail: y2 = relu(p) + exp(min(p,0)) - 1 --
                pb = opool.tile([K, B], F32, tag="elu_p")
                nc.scalar.activation(
                    pb[:, :], ys2pre[:, :], AF.Identity, bias=b2c_sb[:, 0:1]
                )
                r = opool.tile([K, B], F32, tag="elu_r")
                nc.scalar.activation(r[:, :], pb[:, :], AF.Relu)
                neg = opool.tile([K, B], F32, tag="elu_n")
                nc.vector.tensor_sub(neg[:, :], pb[:, :], r[:, :])
                e = opool.tile([K, B], F32, tag="elu_e")
                nc.scalar.activation(e[:, :], neg[:, :], AF.Exp)
                s = opool.tile([K, B], F32, tag="elu_s")
                nc.vector.tensor_add(s[:, :], r[:, :], e[:, :])
                y2f = opool.tile([K, B], F32, tag="elu_y")
                nc.vector.tensor_scalar_add(y2f[:, :], s[:, :], -1.0)
                nc.sync.dma_start(ys2[:, :], y2f[:, :])

    _split_waits(nc)
    return nc


def make_in_map(initial, encoder_hidden, encoder_cell, Wx, Wu, b, w1, b1, w2, b2):
    """Per-core input dict from this core's batch shard (numpy fp32 arrays)."""
    import ml_dtypes
    bf = lambda a: np.ascontiguousarray(a).astype(ml_dtypes.bfloat16)
    f32 = lambda a: np.ascontiguousarray(a, dtype=np.float32)
    return {
        "hT0": bf(encoder_hidden.T),
        "cT0": f32(encoder_cell.T),
        "wu": bf(Wu),
        "wx": bf(np.broadcast_to(Wx, (4, G))),
        "w12": bf(np.concatenate([w1, w2], axis=1)),
        "bvec": f32(b),
        "b12": np.array([[np.float32(b1[0])], [np.float32(b2[0])]], dtype=np.float32),
        "b2col": np.full((K, 1), np.float32(b2[0]), dtype=np.float32),
        "x0": bf(initial[:, 0, :].T),
    }


_CACHE = {}


def _get_nc():
    if "nc" not in _CACHE:
        _CACHE["nc"] = build_nc(repeat=0, rowtile_wx=True)
    return _CACHE["nc"]


def kernel(initial, encoder_hidden, encoder_cell, Wx, Wu, b, w1, b1, w2, b2):
    from concourse import bass_utils

    initial = np.asarray(initial, dtype=np.float32)
    encoder_hidden = np.asarray(encoder_hidden, dtype=np.float32)
    encoder_cell = np.asarray(encoder_cell, dtype=np.float32)
    Wx = np.asarray(Wx, dtype=np.float32)
    Wu = np.asarray(Wu, dtype=np.float32)
    b = np.asarray(b, dtype=np.float32)
    w1 = np.asarray(w1, dtype=np.float32)
    b1 = np.asarray(b1, dtype=np.float32)
    w2 = np.asarray(w2, dtype=np.float32)
    b2 = np.asarray(b2, dtype=np.float32)

    nc = _get_nc()
    in_maps = []
    for c in range(NCORES):
        sl = slice(c * B, (c + 1) * B)
        in_maps.append(
            make_in_map(initial[sl], encoder_hidden[sl], encoder_cell[sl],
                        Wx, Wu, b, w1, b1, w2, b2)
        )
    res = bass_utils.run_bass_kernel_spmd(nc, in_maps, core_ids=list(range(NCORES)))
    out1 = np.concatenate([res.results[c]["ys1"].T for c in range(NCORES)], axis=0)
    out2 = np.concatenate([res.results[c]["ys2"].T for c in range(NCORES)], axis=0)
    return (np.ascontiguousarray(out1, dtype=np.float32),
            np.ascontiguousarray(out2, dtype=np.float32))
